# revision 35
# baseline (speedup 1.0000x reference)
"""Neural-HMM (OverFlow-style) Trainium2 Bass kernel.

Data-parallel over batch B=8 across 8 NeuronCores (one batch element per
core). Per core the T=256-step scan runs fully unrolled:

  - Precompute (once): prenet over all T, LSTM input-to-hidden gates for
    all T, outputnet first-layer encoder term U = inputs @ ow0_enc.T + b.
  - Per step: recurrent matvec [w_hh; ow0_h] @ h_t as a PE row-stream
    (f32r), PSUM->SBUF DMA reshape + PE transpose to get gates in
    partition-major layout, LSTM cell elementwise, outputnet layers 2/3
    (bf16 matmuls), Gaussian emission via a PE reduction, and the HMM
    forward recursion in log space on (1,128)-shaped tiles.

All transcendentals are built from {exp, ln, abs, relu} so the whole
kernel stays inside the single `natural_log_exp_and_others` ACT table
set (sigmoid/tanh/softplus live in other sets; switching sets costs
~2.7us per switch). sigmoid(x) = 1/(1+exp(-x)) with the reciprocal on
the vector engine; tanh(x) = 2*sigmoid(2x)-1 with the 2x folded into
the LSTM g-gate weights on the host.

All shapes/constants are hardcoded for the graded problem size.
"""

import os

import numpy as np
import ml_dtypes

import concourse.bass as bass
import concourse.mybir as mybir
from concourse.tile import TileContext, ScopedClock
from concourse.bass_utils import run_bass_kernel_spmd

F32 = mybir.dt.float32
F32R = mybir.dt.float32r
BF16 = mybir.dt.bfloat16
AF = mybir.ActivationFunctionType
ALU = mybir.AluOpType
AX = mybir.AxisListType

B, N, E, D, T, M, P = 8, 128, 512, 80, 256, 512, 256
NEG = -1e10
LN_EPS = -9.210340371976182  # ln(1e-4)
EM_CONST = -0.5 * D * float(np.log(2.0 * np.pi))
T_STEPS = int(os.environ.get("HMM_T_STEPS", str(T)))
RING_R = 16


def _patch_tile_drain():
    """walrus in this container rejects >1 sync-wait on the kernel-tail
    drain; split the waits across SP nops."""
    if getattr(TileContext, "_drain_patched", False):
        return

    def _drain_and_barrier(self, tick_clock, wait_clock):
        drain_inst = self.nc.sync.drain()
        wait_clock.add_sem_waits(
            drain_inst.ins, ScopedClock({None: tick_clock.global_clock})
        )
        si = drain_inst.ins.sync_info
        waits = list(si.on_wait or [])
        if len(waits) > 1:
            si.on_wait = waits[:1]
            for w in waits[1:]:
                nop = self.nc.sync.nop()
                nsi = nop.ins.sync_info
                if nsi is None:
                    nop.ins.sync_info = mybir.SyncInfo(on_wait=[w], on_update=[])
                else:
                    nsi.on_wait = [w]
        self.nc.all_engine_barrier()
        popped = self.nc._tile_sem_poison_stack.pop()
        assert popped is self._sem_poison
        self.nc.clear_and_free_semaphores(list(self.sems.allocated().values()))
        self.nc.all_engine_barrier()

    TileContext._drain_and_barrier = _drain_and_barrier
    TileContext._drain_patched = True


def _split_multi_waits(nc, maxw=1):
    """walrus here allows only `maxw` sync-waits per instruction; hoist the
    extras onto same-engine NoOps placed immediately before."""
    idx = 0
    for f in nc.m.functions:
        for bb in f.blocks:
            new_list = []
            changed = False
            for inst in bb.instructions:
                si = inst.sync_info
                waits = list(si.on_wait) if (si and si.on_wait) else []
                if len(waits) > maxw:
                    changed = True
                    for i in range(maxw, len(waits), maxw):
                        nop = mybir.InstNoOp(name=f"wsplit-{idx}", ins=[],
                                             outs=[])
                        idx += 1
                        nop.engine = inst.engine
                        nop.sync_info = mybir.SyncInfo(
                            on_wait=waits[i:i + maxw], on_update=[])
                        new_list.append(nop)
                    si.on_wait = waits[:maxw]
                new_list.append(inst)
            if changed:
                bb.instructions = new_list


def build_nc(t_steps: int):
    _patch_tile_drain()
    nc = bass.Bass(target_bir_lowering=False)
    ring_c = (t_steps + RING_R - 1) // RING_R

    dram = {}

    def din(name, shape, dtype=F32):
        dram[name] = nc.dram_tensor(name, list(shape), dtype, kind="ExternalInput")

    din("mels_b", (D, T))
    din("inpT_b", (E, N))
    din("mel_mask", (1, t_steps))
    din("w_catT", (M, 2048), BF16)
    din("ow0hT", (E, P), F32R)
    din("w0T", (D, P), F32R)
    din("w1T", (P, P), F32R)
    din("w_ihT", (P, 4 * M), F32R)
    din("ow0eT", (E, P))
    din("ow1T", (P, P))
    din("ow2T", (P, 226))
    din("ob0t", (128, 2))
    din("ob1t", (128, 2))
    din("b_rt", (128, 16))
    din("b_sp", (D, 1))
    din("ob2mean", (D, 1))
    din("exq_s", (98, 1))
    din("exq_b", (98, 1))
    din("priors", (1, N))
    din("cap_v", (1, N))
    din("ident", (128, 128))
    din("em_wa", (97, 1))
    din("em_wb", (D, 1))

    la_out = nc.dram_tensor("la_out", [t_steps, N], F32, kind="ExternalOutput")
    lp_out = nc.dram_tensor("lp_out", [1, 1], F32, kind="ExternalOutput")

    with TileContext(nc) as tc:
        with tc.tile_pool(name="cst", bufs=1) as cst:
            def load(name, shape, dtype=F32, src=None, tag=None):
                t = cst.tile(list(shape), dtype, tag=tag or name, name=tag or name)
                nc.sync.dma_start(
                    out=t[:], in_=(src if src is not None else dram[name])[:])
                return t

            wcat = [load("w_catT", (128, 2048), BF16,
                         dram["w_catT"][128 * k:128 * (k + 1), :], f"wcat{k}")
                    for k in range(4)]
            ow0h_s = [load("ow0hT", (128, 256), F32R,
                           dram["ow0hT"][128 * k:128 * (k + 1), :], f"ow0h{k}")
                      for k in range(4)]
            wih = [load("w_ihT", (128, 2048), F32R,
                        dram["w_ihT"][128 * k:128 * (k + 1), :], f"wih{k}")
                   for k in range(2)]
            w0t_s = load("w0T", (D, 256), F32R)
            w1t_s = [load("w1T", (128, 256), F32R,
                          dram["w1T"][128 * k:128 * (k + 1), :], f"w1t{k}")
                     for k in range(2)]
            ow0e_s = [load("ow0eT", (128, 256), F32,
                           dram["ow0eT"][128 * k:128 * (k + 1), :], f"ow0e{k}")
                      for k in range(4)]
            ow1_s = [load("ow1T", (128, 256), F32,
                          dram["ow1T"][128 * k:128 * (k + 1), :], f"ow1s{k}")
                     for k in range(2)]
            ow2_s = [load("ow2T", (128, 226), F32,
                          dram["ow2T"][128 * k:128 * (k + 1), :], f"ow2s{k}")
                     for k in range(2)]
            mels_s = load("mels_b", (D, T))
            inpT_s = [load("inpT_b", (128, N), F32,
                           dram["inpT_b"][128 * k:128 * (k + 1), :], f"inpT{k}")
                      for k in range(4)]
            ob0_s = load("ob0t", (128, 2))
            ob1_s = load("ob1t", (128, 2))
            brt_s = load("b_rt", (128, 16))
            bsp_s = load("b_sp", (D, 1))
            ob2m_s = load("ob2mean", (D, 1))
            exqs_s = load("exq_s", (98, 1))
            exqb_s = load("exq_b", (98, 1))
            priors_s = load("priors", (1, N))
            cap_s = load("cap_v", (1, N))
            ident_s = load("ident", (128, 128))
            emwa_s = load("em_wa", (97, 1))
            emwb_s = load("em_wb", (D, 1))
            mask_s = load("mel_mask", (1, t_steps))

            g_ih = cst.tile([128, t_steps, 16], F32, tag="g_ih")
            U_sb = cst.tile([128, 2, 128], F32, tag="U_sb")
            xmb = cst.tile([D, T], F32, tag="xmb")
            la_buf = [cst.tile([1, RING_R * N], F32, tag=f"la_buf{i}",
                               name=f"la_buf{i}") for i in range(2)]
            lc_buf = cst.tile([1, t_steps], F32, tag="lc_buf")
            msk_t = cst.tile([1, t_steps], F32, tag="msk_t")
            lp1 = cst.tile([1, 1], F32, tag="lp1")
            h_sb = cst.tile([128, 4], F32R, tag="h_sb")
            h_bf = cst.tile([128, 4], BF16, tag="h_bf")
            c_sb = cst.tile([128, 4], F32, tag="c_sb")
            la_row = cst.tile([1, N], F32, tag="la_row")
            stack_a = cst.tile([97, N], F32, tag="stack_a")
            tvb_t = cst.tile([1, N], F32, tag="tvb_t")
            sp_tv = cst.tile([1, N], F32, tag="sp_tv")
            stack_b = cst.tile([D, N], F32, tag="stack_b")
            d_t = cst.tile([1, N], F32, tag="d_t")
            m_t = cst.tile([1, N], F32, tag="m_t")
            lq_t = cst.tile([1, N], F32, tag="lq_t")
            ad_t = cst.tile([1, N], F32, tag="ad_t")
            eq_t = cst.tile([1, N], F32, tag="eq_t")
            u3_t = cst.tile([1, N], F32, tag="u3_t")
            sl = cst.tile([1, 2 * N], F32, tag="sl")
            exq_t = cst.tile([98, N], F32, tag="exq_t")
            uq_t = cst.tile([98, N], F32, tag="uq_t")
            spq_t = cst.tile([98, N], F32, tag="spq_t")
            g_sb = cst.tile([128, 16], F32, tag="g_sb")
            ex16 = cst.tile([128, 16], F32, tag="ex16")
            u16 = cst.tile([128, 16], F32, tag="u16")
            r16 = cst.tile([128, 16], F32, tag="r16")
            tg = cst.tile([128, 4], F32, tag="tg")
            exc = cst.tile([128, 4], F32, tag="exc")
            uc = cst.tile([128, 4], F32, tag="uc")
            rc = cst.tile([128, 4], F32, tag="rc")
            t1 = cst.tile([128, 4], F32, tag="t1")
            t2 = cst.tile([128, 4], F32, tag="t2")
            t3 = cst.tile([128, 4], F32, tag="t3")
            v4_sb = cst.tile([97, 512], F32, tag="v4_sb")
            vp_sb = cst.tile([1, 256], F32, tag="vp_sb")
            lp_sb = cst.tile([1, 1], F32, tag="lp_sb")
            z_sb = cst.tile([D, N], F32, tag="z_sb")
            rstd = cst.tile([D, N], F32, tag="rstd")
            c915 = cst.tile([D, 1], F32, tag="c915")
            e_sb = cst.tile([1, N], F32, tag="e_sb")
            la_t = cst.tile([1, N], F32, tag="la_t")
            ngm = cst.tile([1, 1], F32, tag="ngm")
            sum_sb = cst.tile([1, 1], F32, tag="sum_sb")
            lns = cst.tile([1, 1], F32, tag="lns")
            a1_bf = cst.tile([128, 2, 128], F32, tag="a1_bf")
            a2_bf = cst.tile([128, 2, 128], F32, tag="a2_bf")
            zeros = cst.tile([128, 128], F32, tag="zeros")
            eps_t = cst.tile([D, 1], F32, tag="eps_t")

            nc.vector.memset(h_sb[:].bitcast(F32), 0.0)
            nc.vector.memset(h_bf[:], 0.0)
            nc.vector.memset(c_sb[:], 0.0)
            nc.vector.memset(d_t[:], 1e10)
            nc.vector.memset(stack_a[64:96, :], 0.0)
            nc.vector.memset(c915[:], 0.9189385332046727)
            nc.vector.memset(zeros[:], 0.0)
            nc.vector.memset(eps_t[:], 2.5066282746310002e-3)

            # ============ precompute ============
            with tc.tile_pool(name="pre_ps", bufs=2, space="PSUM") as pps, \
                 tc.tile_pool(name="pre_sb", bufs=2) as psb:
                nc.vector.tensor_scalar(out=xmb[:], in0=mels_s[:],
                                        scalar1=ob2m_s[:, 0:1], scalar2=None,
                                        op0=ALU.subtract)
                ar = psb.tile([D, T], F32R, tag="ar")
                nc.vector.memset(ar[:, 0:1].bitcast(F32), 0.0)
                nc.vector.tensor_copy(ar[:, 1:T], mels_s[:, 0:T - 1])

                pre1 = [psb.tile([128, T], F32R, tag=f"pre1_{m}", name=f"pre1_{m}") for m in range(2)]
                for m in range(2):
                    q = pps.tile([128, T], F32, tag="q")
                    nc.tensor.matmul(q[:],
                                     w0t_s[:, 128 * m:128 * (m + 1)],
                                     ar[:], start=True, stop=True)
                    nc.scalar.activation(out=pre1[m][:], in_=q[:], func=AF.Relu)
                pre2 = [psb.tile([128, T], F32R, tag=f"pre2_{m}", name=f"pre2_{m}") for m in range(2)]
                for m in range(2):
                    q = pps.tile([128, T], F32, tag="q")
                    for k in range(2):
                        nc.tensor.matmul(
                            q[:], w1t_s[k][:, 128 * m:128 * (m + 1)],
                            pre1[k][:], start=(k == 0), stop=(k == 1))
                    nc.scalar.activation(out=pre2[m][:], in_=q[:], func=AF.Relu)
                for g in range(16):
                    q = pps.tile([128, T], F32, tag="q")
                    for k in range(2):
                        nc.tensor.matmul(
                            q[:], wih[k][:, 128 * g:128 * (g + 1)],
                            pre2[k][:], start=(k == 0), stop=(k == 1))
                    nc.scalar.activation(out=g_ih[:, 0:t_steps, g],
                                         in_=q[:, 0:t_steps], func=AF.Identity,
                                         bias=brt_s[:, g:g + 1])
                for m in range(2):
                    q = pps.tile([128, N], F32, tag="qU")
                    for k in range(4):
                        nc.tensor.matmul(q[:],
                                         ow0e_s[k][:, 128 * m:128 * (m + 1)],
                                         inpT_s[k][:], start=(k == 0), stop=(k == 3))
                    nc.scalar.activation(out=U_sb[:, m, :], in_=q[:],
                                         func=AF.Identity, bias=ob0_s[:, m:m + 1])

            # ============ scan ============
            with tc.tile_pool(name="v_ps", bufs=2, space="PSUM") as vpool, \
                 tc.tile_pool(name="t_ps", bufs=2, space="PSUM") as tpool, \
                 tc.tile_pool(name="m_ps", bufs=2, space="PSUM") as mpool:
                # 3 column-groups at PSUM partitions {0,32,64}; group q holds
                # v[768q : 768(q+1)] so the 3 row-streams can overlap on PE.
                v4_ps = vpool.tile([128, 512], F32, tag="v")
                vT_ps = tpool.tile([128, 4, 97], F32, tag="vT")
                vp_ps = tpool.tile([1, 256], F32, tag="vp", bufs=1)
                vpT_ps = tpool.tile([128, 2], F32, tag="vpT", bufs=1)

                for t in range(t_steps):
                    r, cc = t % RING_R, t // RING_R

                    # LSTM gates: g_ih[t] + w_hh @ h_t (vT_ps from prev iter)
                    if t == 0:
                        nc.scalar.activation(out=ex16[:], in_=g_ih[:, 0, :],
                                             func=AF.Exp, scale=-1.0)
                    else:
                        nc.vector.tensor_add(
                            g_sb[:],
                            vT_ps[:, 0:4, 0:97:32].rearrange(
                                "p b q -> p q b"),
                            g_ih[:, t, :])
                        nc.scalar.activation(out=ex16[:], in_=g_sb[:],
                                             func=AF.Exp, scale=-1.0)
                    nc.gpsimd.tensor_scalar(out=u16[:], in0=ex16[:], scalar1=1.0,
                                            scalar2=None, op0=ALU.add)
                    nc.vector.reciprocal(out=r16[:], in_=u16[:])
                    nc.gpsimd.tensor_scalar(out=tg[:], in0=r16[:, 12:16],
                                            scalar1=2.0, scalar2=1.0,
                                            op0=ALU.mult, op1=ALU.subtract)
                    nc.vector.tensor_tensor(out=t1[:], in0=r16[:, 4:8],
                                            in1=c_sb[:], op=ALU.mult)
                    nc.vector.tensor_mul(t2[:], r16[:, 0:4], tg[:])
                    nc.vector.tensor_add(c_sb[:], t1[:], t2[:])
                    nc.scalar.activation(out=exc[:], in_=c_sb[:], func=AF.Exp,
                                         scale=-2.0)
                    nc.gpsimd.tensor_scalar(out=uc[:], in0=exc[:], scalar1=1.0,
                                            scalar2=None, op0=ALU.add)
                    nc.vector.reciprocal(out=rc[:], in_=uc[:])
                    nc.vector.tensor_mul(t3[:], r16[:, 8:12], rc[:])
                    nc.vector.scalar_tensor_tensor(out=h_sb[:], in0=t3[:],
                                                   scalar=2.0, in1=r16[:, 8:12],
                                                   op0=ALU.mult, op1=ALU.subtract)
                    nc.vector.scalar_tensor_tensor(out=h_bf[:], in0=t3[:],
                                                   scalar=2.0, in1=r16[:, 8:12],
                                                   op0=ALU.mult, op1=ALU.subtract)

                    # recurrent matvec: gates = w_hh @ h (bf16, 4 col
                    # groups of 512), vproj = ow0h @ h (f32r at group 0)
                    for k in range(4):
                        for q in range(4):
                            nc.tensor.matmul(
                                v4_ps[32 * q:32 * q + 1, :],
                                h_bf[:, k:k + 1],
                                wcat[k][:, 512 * q:512 * (q + 1)],
                                start=(k == 0), stop=(k == 3),
                                tile_position=(0, 32 * q))
                        nc.tensor.matmul(vp_ps[0:1, :], h_sb[:, k:k + 1],
                                         ow0h_s[k][:], start=(k == 0),
                                         stop=(k == 3))
                    nc.scalar.activation(out=v4_sb[:], in_=v4_ps[0:97, :],
                                         func=AF.Identity)
                    nc.scalar.activation(out=vp_sb[:], in_=vp_ps[0:1, :],
                                         func=AF.Identity)
                    for b in range(4):
                        nc.tensor.transpose(vT_ps[:, b, :],
                                            v4_sb[:, 128 * b:128 * (b + 1)],
                                            ident_s[0:97, 0:97])
                    for f in range(2):
                        nc.tensor.transpose(vpT_ps[:, f:f + 1],
                                            vp_sb[0:1, 128 * f:128 * (f + 1)],
                                            ident_s[0:1, 0:1])

                    # outputnet
                    for f in range(2):
                        nc.vector.scalar_tensor_tensor(
                            out=a1_bf[:, f, :], in0=U_sb[:, f, :],
                            scalar=vpT_ps[:, f:f + 1], in1=zeros[:],
                            op0=ALU.add, op1=ALU.max)
                    q1 = [mpool.tile([128, N], F32, tag="mm", name="q1") for _ in range(2)]
                    for f in range(2):
                        for k in range(2):
                            nc.tensor.matmul(q1[f][:],
                                             ow1_s[k][:, 128 * f:128 * (f + 1)],
                                             a1_bf[:, k, :], start=(k == 0),
                                             stop=(k == 1))
                        nc.scalar.activation(out=a2_bf[:, f, :], in_=q1[f][:],
                                             func=AF.Relu,
                                             bias=ob1_s[:, f:f + 1])
                    p0 = mpool.tile([128, N], F32, tag="mm")
                    p1 = mpool.tile([128, N], F32, tag="mm")
                    for k in range(2):
                        nc.tensor.matmul(p0[:], ow2_s[k][:, 0:128], a2_bf[:, k, :],
                                         start=(k == 0), stop=(k == 1))
                    for k in range(2):
                        nc.tensor.matmul(p1[0:98, :], ow2_s[k][:, 128:226],
                                         a2_bf[:, k, :], start=(k == 0),
                                         stop=(k == 1))

                    # merged exp over [std_hat(80) | zeros | tv]; then
                    # softplus = ln(1+e); ln_std' = ln(sqrt(2pi)(sp+1e-3))
                    # folds the -D/2*ln(2pi) emission constant.
                    nc.scalar.activation(out=exq_t[:], in_=p1[0:98, :],
                                         func=AF.Exp, bias=exqb_s[:, 0:1],
                                         scale=exqs_s[:, 0:1])
                    nc.gpsimd.tensor_scalar(out=uq_t[:], in0=exq_t[:],
                                            scalar1=1.0, scalar2=None,
                                            op0=ALU.add)
                    nc.scalar.activation(out=spq_t[:], in_=uq_t[:], func=AF.Ln)
                    nc.scalar.activation(out=stack_b[:], in_=spq_t[0:80, :],
                                         func=AF.Ln,
                                         scale=2.5066282746310002,
                                         bias=eps_t[:, 0:1])
                    nc.scalar.activation(out=rstd[:], in_=stack_b[:], func=AF.Exp,
                                         scale=-1.0, bias=c915[:, 0:1])
                    nc.vector.scalar_tensor_tensor(out=z_sb[:], in0=p0[0:80, :],
                                                   scalar=xmb[:, t:t + 1],
                                                   in1=rstd[:], op0=ALU.subtract,
                                                   op1=ALU.mult)
                    nc.vector.tensor_tensor(out=stack_a[0:80, :], in0=z_sb[:],
                                            in1=z_sb[:], op=ALU.mult)

                    # transition
                    if t == 0:
                        nc.vector.tensor_copy(stack_a[96:97, :], priors_s[:])
                    else:
                        # stay = la - softplus(tvb); move = stay + tvb
                        # (softplus(-x) = softplus(x) - x)
                        nc.vector.tensor_scalar(out=tvb_t[:], in0=p1[96:97, :],
                                                scalar1=exqb_s[96:97, 0:1],
                                                scalar2=None, op0=ALU.add)
                        nc.gpsimd.tensor_copy(sp_tv[:], spq_t[96:97, :])
                        nc.vector.scalar_tensor_tensor(
                            out=sl[0:1, 0:N], in0=sp_tv[:], scalar=-1.0,
                            in1=la_row[:], op0=ALU.mult, op1=ALU.add)
                        nc.vector.tensor_tensor(out=sl[0:1, N:2 * N],
                                                in0=sl[0:1, 0:N], in1=tvb_t[:],
                                                op=ALU.add)
                        nc.vector.tensor_tensor(out=m_t[0:1, 1:N],
                                                in0=sl[0:1, 1:N],
                                                in1=sl[0:1, N:2 * N - 1],
                                                op=ALU.max)
                        nc.gpsimd.tensor_copy(m_t[0:1, 0:1], sl[0:1, 0:1])
                        nc.vector.tensor_tensor(out=d_t[0:1, 1:N],
                                                in0=sl[0:1, 1:N],
                                                in1=sl[0:1, N:2 * N - 1],
                                                op=ALU.subtract)
                        nc.scalar.activation(out=ad_t[:], in_=d_t[:], func=AF.Abs)
                        nc.scalar.activation(out=eq_t[:], in_=ad_t[:], func=AF.Exp,
                                             scale=-1.0)
                        nc.gpsimd.tensor_scalar(out=u3_t[:], in0=eq_t[:],
                                                scalar1=1.0, scalar2=None,
                                                op0=ALU.add)
                        nc.scalar.activation(out=lq_t[:], in_=u3_t[:], func=AF.Ln)
                        nc.vector.tensor_tensor(out=stack_a[96:97, :],
                                                in0=m_t[:], in1=lq_t[:],
                                                op=ALU.add)

                    # em + trans via PE reduction -> la_raw (duplicated rows)
                    em_ps = mpool.tile([1, N], F32, tag="mm")
                    nc.tensor.matmul(em_ps[:], emwa_s[:], stack_a[:],
                                     start=True, stop=False)
                    nc.tensor.matmul(em_ps[:], emwb_s[:], stack_b[:],
                                     start=False, stop=True)

                    # logsumexp + normalize
                    nc.vector.tensor_tensor(out=la_t[:], in0=em_ps[0:1, :],
                                            in1=cap_s[:], op=ALU.min)
                    nc.vector.tensor_reduce(out=ngm[:], in_=la_t[:], axis=AX.X,
                                            op=ALU.max, negate=True)
                    nc.scalar.activation(out=e_sb[:], in_=la_t[:], func=AF.Exp,
                                         bias=ngm[:, 0:1])
                    nc.vector.tensor_reduce(out=sum_sb[:], in_=e_sb[:], axis=AX.X,
                                            op=ALU.add)
                    nc.scalar.activation(out=lns[:], in_=sum_sb[:], func=AF.Ln)
                    nc.vector.tensor_tensor(out=lc_buf[0:1, t:t + 1],
                                            in0=lns[0:1, :], in1=ngm[0:1, :],
                                            op=ALU.subtract)
                    lc_ap = lc_buf[0:1, t:t + 1]
                    buf = la_buf[(t // RING_R) % 2]
                    nc.gpsimd.tensor_scalar(
                        out=buf[0:1, r * N:(r + 1) * N],
                        in0=la_t[:], scalar1=lc_ap, scalar2=None,
                        op0=ALU.subtract)
                    nc.gpsimd.tensor_scalar(out=la_row[:],
                                            in0=la_t[:], scalar1=lc_ap,
                                            scalar2=None, op0=ALU.subtract)
                    if r == RING_R - 1 or t == t_steps - 1:
                        t0c = t - r
                        nc.sync.dma_start(
                            out=la_out[t0c:t + 1, :],
                            in_=buf[0:1, 0:(r + 1) * N].rearrange(
                                "a (rr n) -> a rr n", n=N))

                # ============ outputs ============
                nc.vector.tensor_mul(msk_t[:], lc_buf[:], mask_s[:])
                nc.vector.tensor_reduce(out=lp1[:], in_=msk_t[:], axis=AX.X,
                                        op=ALU.add)
                nc.sync.dma_start(out=lp_out[:], in_=lp1[:])

    _split_multi_waits(nc)
    return nc


_NC_CACHE = {}


def _get_nc(t_steps):
    if t_steps not in _NC_CACHE:
        _NC_CACHE[t_steps] = build_nc(t_steps)
    return _NC_CACHE[t_steps]


def prepare_inputs(inputs, t_steps=T_STEPS):
    """Host-side data prep: layout/dtype only (transposes, reorders, casts)."""
    f32 = np.float32
    inp = {k: np.asarray(v) for k, v in inputs.items()}
    # gate reorder [i,f,g,o] -> [i,f,o,g]; double the g-gate rows (tanh trick)
    perm = np.r_[0:512, 512:1024, 1536:2048, 1024:1536]
    gscale = np.ones((2048, 1), f32)
    gscale[1536:2048] = 2.0
    w_hh_r = inp["lstm_w_hh"][perm] * gscale
    w_ih_r = inp["lstm_w_ih"][perm] * gscale
    b_r = ((inp["lstm_b_ih"] + inp["lstm_b_hh"])[perm].reshape(2048, 1)
           * gscale).reshape(2048).astype(f32)

    ow0e = inp["ow0"][:, :E]
    ow0h = inp["ow0"][:, E:]

    ow2 = inp["ow2"]
    ob2 = np.asarray(inp["ob2"], f32)
    ow2T = np.zeros((P, 226), f32)
    ow2T[:, 0:80] = ow2[0:80].T
    ow2T[:, 128:208] = ow2[80:160].T
    ow2T[:, 224] = ow2[160]
    b_tv = float(ob2[160])

    em_wa = np.zeros((97, 1), f32)
    em_wa[0:80, :] = -0.5
    em_wa[96, :] = 1.0
    em_wb = np.full((D, 1), -1.0, f32)

    exq_s = np.zeros((98, 1), f32)
    exq_s[0:80] = 1.0
    exq_s[96] = 1.0
    exq_b = np.zeros((98, 1), f32)
    exq_b[0:80] = ob2[80:160].reshape(80, 1)
    exq_b[96] = b_tv

    priors = np.full((1, N), NEG, f32)
    priors[0, 0] = 0.0

    shared = {
        "w_catT": np.ascontiguousarray(w_hh_r.T).astype(ml_dtypes.bfloat16),
        "ow0hT": np.ascontiguousarray(ow0h.T, dtype=f32),
        "w0T": np.ascontiguousarray(inp["prenet_w0"].T, dtype=f32),
        "w1T": np.ascontiguousarray(inp["prenet_w1"].T, dtype=f32),
        "w_ihT": np.ascontiguousarray(w_ih_r.T, dtype=f32),
        "ow0eT": np.ascontiguousarray(ow0e.T, dtype=f32),
        "ow1T": np.ascontiguousarray(inp["ow1"].T, dtype=f32),
        "ow2T": ow2T,
        "ob0t": np.ascontiguousarray(np.asarray(inp["ob0"], f32).reshape(2, 128).T),
        "ob1t": np.ascontiguousarray(np.asarray(inp["ob1"], f32).reshape(2, 128).T),
        "b_rt": np.ascontiguousarray(b_r.reshape(16, 128).T),
        "b_sp": np.ascontiguousarray(ob2[80:160].reshape(D, 1)),
        "ob2mean": np.ascontiguousarray(ob2[0:80].reshape(D, 1)),
        "exq_s": exq_s,
        "exq_b": exq_b,
        "priors": priors,
        "ident": np.eye(128, dtype=f32),
        "em_wa": em_wa,
        "em_wb": em_wb,
    }

    mel_lens = np.asarray(inp["mel_lens"]).astype(np.int64)
    inputs_len = np.asarray(inp["inputs_len"]).astype(np.int64)
    in_maps = []
    for b in range(B):
        mm = (np.arange(t_steps) < mel_lens[b]).astype(f32).reshape(1, t_steps)
        cap = np.where(np.arange(N) < inputs_len[b], 3.0e38, NEG)
        m = dict(shared)
        m["cap_v"] = cap.astype(f32).reshape(1, N)
        m["mels_b"] = np.ascontiguousarray(inp["mels"][b], dtype=f32)
        m["inpT_b"] = np.ascontiguousarray(inp["inputs"][b].T, dtype=f32)
        m["mel_mask"] = mm
        in_maps.append(m)
    return in_maps


def kernel(**inputs):
    t_steps = T_STEPS
    nc = _get_nc(t_steps)
    in_maps = prepare_inputs(inputs, t_steps)
    res = run_bass_kernel_spmd(nc, in_maps, core_ids=list(range(B)))
    log_prob = np.array([res.results[b]["lp_out"][0, 0] for b in range(B)],
                        np.float32)
    la = np.stack([res.results[b]["la_out"] for b in range(B)]).astype(np.float32)
    return log_prob, la


if __name__ == "__main__":
    import reference

    inputs = {k: np.asarray(v) for k, v in reference.setup_inputs().items()}
    lp, la = kernel(**inputs)
    print("lp:", lp)
    print("la[0,0,:8]:", la[0, 0, :8])
